# revision 29
# baseline (speedup 1.0000x reference)
"""Multi-head attention (B=8, N=1024, C=768, 12 heads) on 8 TRN2 NeuronCores.

Sharding: data-parallel over batch — batch element b runs on core b, weights
replicated, zero collectives.

Per-core kernel (all matmuls bf16 on the TensorEngine):
  - Host pre-transposes x, W_qkv, W_proj so every contraction has its
    reduction axis on SBUF partitions; no on-device transposes needed.
  - qkv: q^T,k^T [768,1024] and v [1024,768] via 6-chunk K=768 matmuls.
  - scores are computed TRANSPOSED per head: S^T[k,q] with lhsT=k^T-block,
    rhs=q^T-block, so the exp output P^T feeds the P@V matmul directly as
    the moving operand (no transpose of the attention matrix). The
    1/sqrt(d) scale rides for free on the exp's affine pre-scale.
  - softmax denominators come free: v is stored with a ones-column
    appended per head (lhsT [128,65]); row 64 of the P@V accumulator is
    sum_k exp(S), i.e. the denominator.
  - normalization runs entirely off the TensorEngine's critical path:
    copy the accumulator to SBUF (releasing its PSUM slot), fast
    approximate reciprocal on VectorE, broadcast across partitions on the
    (otherwise idle) GpSimd engine, one elementwise multiply per head.
  - proj: y = attn @ W_proj^T + b_proj, bias added via a K=1 matmul
    (lhsT=ones row, rhs=bias row) accumulated into the same PSUM.
  - qkv chunk emission is interleaved between attention head pairs so the
    TensorEngine stays dense while ScalarE works through the exps.
"""

from contextlib import ExitStack

import numpy as np

import concourse.mybir as mybir
import concourse.tile as tile
from concourse import bacc
from concourse.bass_utils import run_bass_kernel_spmd

B, N, C = 8, 1024, 768
NH, D = 12, 64
CK = C // 128  # 6 contraction chunks of 128
NQ = N // 128  # 8 position chunks of 128
SCALE = D ** -0.5
F32 = mybir.dt.float32
BF16 = mybir.dt.bfloat16
Copy = mybir.ActivationFunctionType.Copy
Exp = mybir.ActivationFunctionType.Exp


def _emit(tc, xT, wqkvT, wprojT, bproj, out):
    nc = tc.nc
    with ExitStack() as ctx:
        sb = ctx.enter_context(tc.tile_pool(name="sb", bufs=1))
        stage = ctx.enter_context(tc.tile_pool(name="stage", bufs=6))
        pp = ctx.enter_context(tc.tile_pool(name="pp", bufs=6))
        small = ctx.enter_context(tc.tile_pool(name="small", bufs=2))
        # PSUM pools are released by hand: qkv+attention use ps/acc, the
        # projection reuses the freed banks for a deeper y pipeline.
        ps = tc.alloc_tile_pool(name="ps", bufs=3, space="PSUM")
        acc = tc.alloc_tile_pool(name="acc", bufs=1, space="PSUM")

        # ---- bias: load row, broadcast across partitions once --------
        bp_row = sb.tile([1, C], F32, name="bp_row", tag="bp_row")
        nc.sync.dma_start(out=bp_row[:], in_=bproj[None, :])
        bias_bc = sb.tile([128, C], F32, name="bias_bc", tag="bias_bc")
        nc.gpsimd.partition_broadcast(bias_bc[:], bp_row[:])

        # ---- load + bf16-convert x^T and W_qkv^T ---------------------
        # The first scores matmul needs all of x^T plus the q-left and
        # k-left weight columns, so those loads are interleaved per
        # c-chunk; v and the right halves follow.
        xT_bf = [
            sb.tile([128, N], BF16, name=f"xT_bf{c}", tag=f"xT_bf{c}")
            for c in range(CK)
        ]
        wq_bf = [
            sb.tile([128, 3 * C], BF16, name=f"wq_bf{c}", tag=f"wq_bf{c}")
            for c in range(CK)
        ]

        def load_w(g, c):
            w_st = stage.tile([128, 384], F32, name=f"w_st{g}_{c}", tag="stage")
            nc.sync.dma_start(
                out=w_st[:],
                in_=wqkvT[c * 128:(c + 1) * 128, g * 384:(g + 1) * 384],
            )
            # convert on ScalarE: it is idle during the qkv phase
            nc.scalar.activation(wq_bf[c][:, g * 384:(g + 1) * 384], w_st[:], Copy)

        def load_x(c, qh):
            x_st = stage.tile([128, 512], F32, name=f"x_st{c}_{qh}", tag="stage")
            nc.sync.dma_start(
                out=x_st[:],
                in_=xT[c * 128:(c + 1) * 128, qh * 512:(qh + 1) * 512],
            )
            nc.vector.tensor_copy(xT_bf[c][:, qh * 512:(qh + 1) * 512], x_st[:])

        for c in range(CK):
            load_x(c, 0)
            load_w(0, c)  # q-left
            load_w(2, c)  # k-left
        for c in range(CK):
            load_x(c, 1)
        for g in (4, 5, 1, 3):  # vL, vR, qR, kR
            for c in range(CK):
                load_w(g, c)

        # ---- qkv projections -----------------------------------------
        # q^T,k^T: chunk m covers rows [m*128,(m+1)*128) of qkv^T;
        # m in 0..5 -> q, m in 6..11 -> k.
        qkT = [
            sb.tile([128, N], BF16, name=f"qkT{m}", tag=f"qkT{m}")
            for m in range(12)
        ]

        def emit_qk(m):
            for qh in range(2):
                qk_ps = ps.tile([128, 512], F32, name=f"qk_ps{m}_{qh}", tag="s")
                for c in range(CK):
                    nc.tensor.matmul(
                        qk_ps[:],
                        lhsT=wq_bf[c][:, m * 128:(m + 1) * 128],
                        rhs=xT_bf[c][:, qh * 512:(qh + 1) * 512],
                        start=(c == 0),
                        stop=(c == CK - 1),
                    )
                nc.vector.tensor_copy(qkT[m][:, qh * 512:(qh + 1) * 512], qk_ps[:])

        # v in natural layout [n, (head, d)] with a ones column appended
        # per head: v_sb[n] is [128, NH, D+1], [:, h, D] == 1.0.
        v_sb = [
            sb.tile([128, NH, D + 1], BF16, name=f"v_sb{n}", tag=f"v_sb{n}")
            for n in range(NQ)
        ]

        def emit_v(n):
            nc.gpsimd.memset(v_sb[n][:, :, D], 1.0)
            for half in range(2):
                v_ps = ps.tile([128, 384], F32, name=f"v_ps{n}_{half}", tag="s")
                for c in range(CK):
                    nc.tensor.matmul(
                        v_ps[:],
                        lhsT=xT_bf[c][:, n * 128:(n + 1) * 128],
                        rhs=wq_bf[c][:, 2 * C + half * 384:2 * C + (half + 1) * 384],
                        start=(c == 0),
                        stop=(c == CK - 1),
                    )
                nc.vector.tensor_copy(
                    v_sb[n][:, half * 6:(half + 1) * 6, 0:D],
                    v_ps[:].rearrange("p (h d) -> p h d", d=D),
                )

        # ---- attention ------------------------------------------------
        attn_bf = [
            sb.tile([128, N], BF16, name=f"attn_bf{p}", tag=f"attn_bf{p}")
            for p in range(6)
        ]

        def emit_head(h, filler=None):
            """S^T + exp + P@V for head h; `filler` emits extra PE work
            early in the stream (previous head's deferred normalize, next
            qkv chunk) so PE has exp-independent work while ScalarE runs."""
            q_tile = qkT[h // 2]
            k_tile = qkT[6 + h // 2]
            ro = (h % 2) * 64
            out_aug = acc.tile([D + 1, N], F32, name=f"oaug{h}", tag="acc")

            def emit_S(kc):
                st = ps.tile([128, N], F32, name=f"s{h}_{kc}", tag="s")
                for qh in range(2):
                    nc.tensor.matmul(
                        st[:, qh * 512:(qh + 1) * 512],
                        lhsT=k_tile[ro:ro + D, kc * 128:(kc + 1) * 128],
                        rhs=q_tile[ro:ro + D, qh * 512:(qh + 1) * 512],
                        start=True,
                        stop=True,
                    )
                pt = pp.tile([128, N], BF16, name=f"P{h}_{kc}", tag="P")
                nc.scalar.activation(pt[:], st[:], Exp, scale=SCALE)
                return pt

            def emit_v_mm(kc, pt):
                for qh in range(2):
                    nc.tensor.matmul(
                        out_aug[:, qh * 512:(qh + 1) * 512],
                        lhsT=v_sb[kc][:, h, :],
                        rhs=pt[:, qh * 512:(qh + 1) * 512],
                        start=(kc == 0),
                        stop=(kc == NQ - 1),
                    )

            # software pipeline: exp(kc) overlaps S(kc+1) and P@V(kc-1)
            pts = {0: emit_S(0), 1: emit_S(1)}
            if filler is not None:
                filler()
            for kc in range(NQ):
                emit_v_mm(kc, pts.pop(kc))
                if kc + 2 < NQ:
                    pts[kc + 2] = emit_S(kc + 2)
            return out_aug

        def emit_norm_pre(h, oa, direct=False):
            """DVE/GpSimd-only part: reciprocal chain first (it gates the
            final multiply), then stage the accumulator to SBUF to release
            its PSUM slot. For the last head (`direct`) the multiply reads
            the accumulator straight from PSUM instead — shortest tail."""
            dn = small.tile([1, N], F32, name=f"dn{h}", tag="dn")
            nc.vector.tensor_copy(dn[:], oa[D:D + 1, :])
            rc = small.tile([1, N], F32, name=f"rc{h}", tag="rc")
            # reciprocal_approx_fast's uOp program only works from
            # partition 0 on hardware, hence the dn bounce copy above.
            nc.vector.reciprocal_approx_fast(rc[:], dn[:])
            rcb = small.tile([1, N], BF16, name=f"rcb{h}", tag="rcb")
            nc.vector.tensor_copy(rcb[:], rc[:])
            bcast = small.tile([64, N], BF16, name=f"bcast{h}", tag="bcast")
            nc.gpsimd.partition_broadcast(bcast[:], rcb[:])
            if direct:
                return oa, bcast
            un = small.tile([D, N], F32, name=f"un{h}", tag="un")
            nc.vector.tensor_copy(un[:], oa[0:D, :])
            return un, bcast

        def emit_norm_post(h, un, bcast):
            p, ro = h // 2, (h % 2) * 64
            nc.vector.tensor_mul(attn_bf[p][ro:ro + 64, :], un[0:D, :], bcast[:])

        emit_qk(0)
        emit_qk(6)
        emit_v(0)

        # Remaining qkv work rides inside the attention stream as PE
        # filler during exp waits: head 0 carries the other v chunks
        # (needed from its own P@V loop onward), later heads each carry
        # one q/k chunk, landing one pair ahead of first use.
        QK_FILL = {1: (1, 7), 2: (2,), 3: (8,), 4: (3,), 5: (9,),
                   6: (4,), 7: (10,), 8: (5,), 9: (11,)}
        pending = None
        for h in range(NH):
            fillers = []
            if h == 0:
                fillers.append(lambda: [emit_v(n) for n in range(1, NQ)])
            for m in QK_FILL.get(h, ()):
                fillers.append(lambda m=m: emit_qk(m))
            if pending is not None:
                ph, un, bc = pending
                fillers.append(lambda ph=ph, un=un, bc=bc: emit_norm_post(ph, un, bc))

            def filler():
                for f in fillers:
                    f()

            oa = emit_head(h, filler=filler)
            pending = (h, *emit_norm_pre(h, oa, direct=(h == NH - 1)))
        emit_norm_post(*pending)

        # ---- output projection ---------------------------------------
        acc.release()
        ps.release()
        yps = tc.alloc_tile_pool(name="yps", bufs=4, space="PSUM")
        wp_bf = []
        for c in range(CK):
            wp_st = stage.tile([128, C], F32, name=f"wp_st{c}", tag="stage")
            nc.sync.dma_start(out=wp_st[:], in_=wprojT[c * 128:(c + 1) * 128, :])
            t = sb.tile([128, C], BF16, name=f"wp_bf{c}", tag=f"wp_bf{c}")
            nc.vector.tensor_copy(t[:], wp_st[:])
            wp_bf.append(t)

        # Two sweeps per group of 4 n-chunks: the c<5 accumulation of the
        # whole group first (runs while head 11's normalize chain is still
        # in flight), then the c=5 closers (which need attn_bf[5]).
        for grp in (range(0, 4), range(4, NQ)):
            y_tiles = {}
            for n in grp:
                y_ps = yps.tile([128, C], F32, name=f"y_ps{n}", tag="y_ps")
                y_tiles[n] = y_ps
                for lo, hi in ((0, 512), (512, 768)):
                    for c in range(CK - 1):
                        nc.tensor.matmul(
                            y_ps[:, lo:hi],
                            lhsT=attn_bf[c][:, n * 128:(n + 1) * 128],
                            rhs=wp_bf[c][:, lo:hi],
                            start=(c == 0),
                            stop=False,
                        )
            for n in grp:
                y_ps = y_tiles[n]
                for lo, hi in ((0, 512), (512, 768)):
                    nc.tensor.matmul(
                        y_ps[:, lo:hi],
                        lhsT=attn_bf[CK - 1][:, n * 128:(n + 1) * 128],
                        rhs=wp_bf[CK - 1][:, lo:hi],
                        start=False,
                        stop=True,
                    )
                y_sb = stage.tile([128, C], F32, name=f"y_sb{n}", tag="y", bufs=2)
                nc.vector.tensor_add(y_sb[:], y_ps[:], bias_bc[:])
                nc.sync.dma_start(out=out[n * 128:(n + 1) * 128, :], in_=y_sb[:])
        yps.release()


def build_graph():
    nc = bacc.Bacc("TRN2", target_bir_lowering=False, debug=False)
    xT = nc.declare_dram_parameter("xT", [C, N], F32, isOutput=False)
    wqkvT = nc.declare_dram_parameter("wqkvT", [C, 3 * C], F32, isOutput=False)
    wprojT = nc.declare_dram_parameter("wprojT", [C, C], F32, isOutput=False)
    bproj = nc.declare_dram_parameter("bproj", [C], F32, isOutput=False)
    out = nc.declare_dram_parameter("out", [N, C], F32, isOutput=True)
    with tile.TileContext(nc) as tc:
        _emit(tc, xT.ap(), wqkvT.ap(), wprojT.ap(), bproj.ap(), out.ap())
    nc.compile()
    return nc


_GRAPH = None


def _get_graph():
    global _GRAPH
    if _GRAPH is None:
        _GRAPH = build_graph()
    return _GRAPH


def make_in_maps(x, W_qkv, W_proj, b_proj):
    x = np.ascontiguousarray(np.asarray(x, dtype=np.float32))
    wqkvT = np.ascontiguousarray(np.asarray(W_qkv, dtype=np.float32).T)
    wprojT = np.ascontiguousarray(np.asarray(W_proj, dtype=np.float32).T)
    bp = np.ascontiguousarray(np.asarray(b_proj, dtype=np.float32))
    xT_all = np.ascontiguousarray(x.transpose(0, 2, 1))
    return [
        {"xT": xT_all[i], "wqkvT": wqkvT, "wprojT": wprojT, "bproj": bp}
        for i in range(B)
    ]


def run(x, W_qkv, W_proj, b_proj, trace=False):
    nc = _get_graph()
    in_maps = make_in_maps(x, W_qkv, W_proj, b_proj)
    res = run_bass_kernel_spmd(nc, in_maps, core_ids=list(range(B)), trace=trace)
    out = np.stack([res.results[i]["out"] for i in range(B)], axis=0)
    return out.astype(np.float32, copy=False), res


def kernel(x, W_qkv, W_proj, b_proj, H=None, W=None):
    out, _ = run(x, W_qkv, W_proj, b_proj)
    return out


# revision 30
# speedup vs baseline: 1.1902x; 1.1902x over previous
"""Multi-head attention (B=8, N=1024, C=768, 12 heads) on 8 TRN2 NeuronCores.

Sharding: data-parallel over batch — batch element b runs on core b, weights
replicated, zero collectives.

Per-core kernel (all matmuls bf16 on the TensorEngine):
  - Host pre-transposes x, W_qkv, W_proj so every contraction has its
    reduction axis on SBUF partitions; no on-device transposes needed.
  - qkv: q^T,k^T [768,1024] and v [1024,768] via 6-chunk K=768 matmuls.
  - scores are computed TRANSPOSED per head: S^T[k,q] with lhsT=k^T-block,
    rhs=q^T-block, so the exp output P^T feeds the P@V matmul directly as
    the moving operand (no transpose of the attention matrix). The
    1/sqrt(d) scale rides for free on the exp's affine pre-scale.
  - softmax denominators come free: v is stored with a ones-column
    appended per head (lhsT [128,65]); row 64 of the P@V accumulator is
    sum_k exp(S), i.e. the denominator.
  - normalization runs entirely off the TensorEngine's critical path:
    copy the accumulator to SBUF (releasing its PSUM slot), fast
    approximate reciprocal on VectorE, broadcast across partitions on the
    (otherwise idle) GpSimd engine, one elementwise multiply per head.
  - proj: y = attn @ W_proj^T + b_proj, bias added via a K=1 matmul
    (lhsT=ones row, rhs=bias row) accumulated into the same PSUM.
  - qkv chunk emission is interleaved between attention head pairs so the
    TensorEngine stays dense while ScalarE works through the exps.
"""

from contextlib import ExitStack

import numpy as np

import concourse.mybir as mybir
import concourse.tile as tile
from concourse import bacc
from concourse.bass_utils import run_bass_kernel_spmd

B, N, C = 8, 1024, 768
NH, D = 12, 64
CK = C // 128  # 6 contraction chunks of 128
NQ = N // 128  # 8 position chunks of 128
SCALE = D ** -0.5
F32 = mybir.dt.float32
BF16 = mybir.dt.bfloat16
Copy = mybir.ActivationFunctionType.Copy
Exp = mybir.ActivationFunctionType.Exp


def _emit(tc, xT, wqkvT, wprojT, bproj, out):
    nc = tc.nc
    with ExitStack() as ctx:
        sb = ctx.enter_context(tc.tile_pool(name="sb", bufs=1))
        stage = ctx.enter_context(tc.tile_pool(name="stage", bufs=6))
        pp = ctx.enter_context(tc.tile_pool(name="pp", bufs=6))
        small = ctx.enter_context(tc.tile_pool(name="small", bufs=2))
        # PSUM pools are released by hand: qkv+attention use ps/acc, the
        # projection reuses the freed banks for a deeper y pipeline.
        ps = tc.alloc_tile_pool(name="ps", bufs=3, space="PSUM")
        acc = tc.alloc_tile_pool(name="acc", bufs=1, space="PSUM")

        # ---- bias: load row, broadcast across partitions once --------
        bp_row = sb.tile([1, C], F32, name="bp_row", tag="bp_row")
        nc.sync.dma_start(out=bp_row[:], in_=bproj[None, :])
        bias_bc = sb.tile([128, C], F32, name="bias_bc", tag="bias_bc")
        nc.gpsimd.partition_broadcast(bias_bc[:], bp_row[:])

        # ---- load + bf16-convert x^T and W_qkv^T ---------------------
        # The first scores matmul needs all of x^T plus the q-left and
        # k-left weight columns, so those loads are interleaved per
        # c-chunk; v and the right halves follow.
        xT_bf = [
            sb.tile([128, N], BF16, name=f"xT_bf{c}", tag=f"xT_bf{c}")
            for c in range(CK)
        ]
        wq_bf = [
            sb.tile([128, 3 * C], BF16, name=f"wq_bf{c}", tag=f"wq_bf{c}")
            for c in range(CK)
        ]

        def load_w(g, c):
            w_st = stage.tile([128, 384], F32, name=f"w_st{g}_{c}", tag="stage")
            nc.sync.dma_start(
                out=w_st[:],
                in_=wqkvT[c * 128:(c + 1) * 128, g * 384:(g + 1) * 384],
            )
            # convert on ScalarE: it is idle during the qkv phase
            nc.scalar.activation(wq_bf[c][:, g * 384:(g + 1) * 384], w_st[:], Copy)

        def load_x(c, qh):
            x_st = stage.tile([128, 512], F32, name=f"x_st{c}_{qh}", tag="stage")
            nc.sync.dma_start(
                out=x_st[:],
                in_=xT[c * 128:(c + 1) * 128, qh * 512:(qh + 1) * 512],
            )
            nc.vector.tensor_copy(xT_bf[c][:, qh * 512:(qh + 1) * 512], x_st[:])

        for c in range(CK):
            load_x(c, 0)
            load_w(0, c)  # q-left
            load_w(2, c)  # k-left
        for c in range(CK):
            load_x(c, 1)
        for g in (4, 5, 1, 3):  # vL, vR, qR, kR
            for c in range(CK):
                load_w(g, c)

        # ---- qkv projections -----------------------------------------
        # q^T,k^T: chunk m covers rows [m*128,(m+1)*128) of qkv^T;
        # m in 0..5 -> q, m in 6..11 -> k.
        qkT = [
            sb.tile([128, N], BF16, name=f"qkT{m}", tag=f"qkT{m}")
            for m in range(12)
        ]

        def emit_qk(m):
            for qh in range(2):
                qk_ps = ps.tile([128, 512], F32, name=f"qk_ps{m}_{qh}", tag="s")
                for c in range(CK):
                    nc.tensor.matmul(
                        qk_ps[:],
                        lhsT=wq_bf[c][:, m * 128:(m + 1) * 128],
                        rhs=xT_bf[c][:, qh * 512:(qh + 1) * 512],
                        start=(c == 0),
                        stop=(c == CK - 1),
                    )
                nc.vector.tensor_copy(qkT[m][:, qh * 512:(qh + 1) * 512], qk_ps[:])

        # v in natural layout [n, (head, d)] with a ones column appended
        # per head: v_sb[n] is [128, NH, D+1], [:, h, D] == 1.0.
        v_sb = [
            sb.tile([128, NH, D + 1], BF16, name=f"v_sb{n}", tag=f"v_sb{n}")
            for n in range(NQ)
        ]

        def emit_v(n):
            nc.gpsimd.memset(v_sb[n][:, :, D], 1.0)
            for half in range(2):
                v_ps = ps.tile([128, 384], F32, name=f"v_ps{n}_{half}", tag="s")
                for c in range(CK):
                    nc.tensor.matmul(
                        v_ps[:],
                        lhsT=xT_bf[c][:, n * 128:(n + 1) * 128],
                        rhs=wq_bf[c][:, 2 * C + half * 384:2 * C + (half + 1) * 384],
                        start=(c == 0),
                        stop=(c == CK - 1),
                    )
                nc.vector.tensor_copy(
                    v_sb[n][:, half * 6:(half + 1) * 6, 0:D],
                    v_ps[:].rearrange("p (h d) -> p h d", d=D),
                )

        # ---- attention ------------------------------------------------
        attn_bf = [
            sb.tile([128, N], BF16, name=f"attn_bf{p}", tag=f"attn_bf{p}")
            for p in range(6)
        ]

        def emit_head(h, filler=None):
            """S^T + exp + P@V for head h; `filler` emits extra PE work
            early in the stream (previous head's deferred normalize, next
            qkv chunk) so PE has exp-independent work while ScalarE runs."""
            q_tile = qkT[h // 2]
            k_tile = qkT[6 + h // 2]
            ro = (h % 2) * 64
            out_aug = acc.tile([D + 1, N], F32, name=f"oaug{h}", tag="acc")

            def emit_S(kc):
                st = ps.tile([128, N], F32, name=f"s{h}_{kc}", tag="s")
                for qh in range(2):
                    nc.tensor.matmul(
                        st[:, qh * 512:(qh + 1) * 512],
                        lhsT=k_tile[ro:ro + D, kc * 128:(kc + 1) * 128],
                        rhs=q_tile[ro:ro + D, qh * 512:(qh + 1) * 512],
                        start=True,
                        stop=True,
                    )
                pt = pp.tile([128, N], BF16, name=f"P{h}_{kc}", tag="P")
                nc.scalar.activation(pt[:], st[:], Exp, scale=SCALE)
                return pt

            def emit_v_mm(kc, pt):
                for qh in range(2):
                    nc.tensor.matmul(
                        out_aug[:, qh * 512:(qh + 1) * 512],
                        lhsT=v_sb[kc][:, h, :],
                        rhs=pt[:, qh * 512:(qh + 1) * 512],
                        start=(kc == 0),
                        stop=(kc == NQ - 1),
                    )

            # software pipeline: exp(kc) overlaps S(kc+1) and P@V(kc-1)
            pts = {0: emit_S(0), 1: emit_S(1)}
            if filler is not None:
                filler()
            for kc in range(NQ):
                emit_v_mm(kc, pts.pop(kc))
                if kc + 2 < NQ:
                    pts[kc + 2] = emit_S(kc + 2)
            return out_aug

        def emit_norm_pre(h, oa, direct=False):
            """DVE/GpSimd-only part: reciprocal chain first (it gates the
            final multiply), then stage the accumulator to SBUF to release
            its PSUM slot. For the last head (`direct`) the multiply reads
            the accumulator straight from PSUM instead — shortest tail."""
            if not direct:
                # staging copy FIRST: it releases the single-slot PSUM
                # accumulator, which gates the next head's P@V matmuls
                un = small.tile([D, N], F32, name=f"un{h}", tag="un")
                nc.vector.tensor_copy(un[:], oa[0:D, :])
            dn = small.tile([1, N], F32, name=f"dn{h}", tag="dn")
            nc.vector.tensor_copy(dn[:], oa[D:D + 1, :])
            rc = small.tile([1, N], F32, name=f"rc{h}", tag="rc")
            # reciprocal_approx_fast's uOp program only works from
            # partition 0 on hardware, hence the dn bounce copy above.
            nc.vector.reciprocal_approx_fast(rc[:], dn[:])
            rcb = small.tile([1, N], BF16, name=f"rcb{h}", tag="rcb")
            nc.vector.tensor_copy(rcb[:], rc[:])
            bcast = small.tile([64, N], BF16, name=f"bcast{h}", tag="bcast")
            nc.gpsimd.partition_broadcast(bcast[:], rcb[:])
            if direct:
                return oa, bcast
            return un, bcast

        def emit_norm_post(h, un, bcast):
            p, ro = h // 2, (h % 2) * 64
            nc.vector.tensor_mul(attn_bf[p][ro:ro + 64, :], un[0:D, :], bcast[:])

        emit_qk(0)
        emit_qk(6)
        emit_v(0)

        # Remaining qkv work rides inside the attention stream as PE
        # filler during exp waits: head 0 carries the other v chunks
        # (needed from its own P@V loop onward), later heads each carry
        # one q/k chunk, landing one pair ahead of first use.
        QK_FILL = {1: (1, 7), 2: (2,), 3: (8,), 4: (3,), 5: (9,),
                   6: (4,), 7: (10,), 8: (5,), 9: (11,)}
        pending = None
        for h in range(NH):
            fillers = []
            if h == 0:
                fillers.append(lambda: [emit_v(n) for n in range(1, NQ)])
            for m in QK_FILL.get(h, ()):
                fillers.append(lambda m=m: emit_qk(m))
            if pending is not None:
                ph, un, bc = pending
                fillers.append(lambda ph=ph, un=un, bc=bc: emit_norm_post(ph, un, bc))

            def filler():
                for f in fillers:
                    f()

            oa = emit_head(h, filler=filler)
            pending = (h, *emit_norm_pre(h, oa, direct=(h == NH - 1)))
        emit_norm_post(*pending)

        # ---- output projection ---------------------------------------
        acc.release()
        ps.release()
        yps = tc.alloc_tile_pool(name="yps", bufs=4, space="PSUM")
        wp_bf = []
        for c in range(CK):
            wp_st = stage.tile([128, C], F32, name=f"wp_st{c}", tag="stage")
            nc.sync.dma_start(out=wp_st[:], in_=wprojT[c * 128:(c + 1) * 128, :])
            t = sb.tile([128, C], BF16, name=f"wp_bf{c}", tag=f"wp_bf{c}")
            nc.vector.tensor_copy(t[:], wp_st[:])
            wp_bf.append(t)

        # Two sweeps per group of 4 n-chunks: the c<5 accumulation of the
        # whole group first (runs while head 11's normalize chain is still
        # in flight), then the c=5 closers (which need attn_bf[5]).
        for grp in (range(0, 4), range(4, NQ)):
            y_tiles = {}
            for n in grp:
                y_ps = yps.tile([128, C], F32, name=f"y_ps{n}", tag="y_ps")
                y_tiles[n] = y_ps
                for lo, hi in ((0, 512), (512, 768)):
                    for c in range(CK - 1):
                        nc.tensor.matmul(
                            y_ps[:, lo:hi],
                            lhsT=attn_bf[c][:, n * 128:(n + 1) * 128],
                            rhs=wp_bf[c][:, lo:hi],
                            start=(c == 0),
                            stop=False,
                        )
            for n in grp:
                y_ps = y_tiles[n]
                for lo, hi in ((0, 512), (512, 768)):
                    nc.tensor.matmul(
                        y_ps[:, lo:hi],
                        lhsT=attn_bf[CK - 1][:, n * 128:(n + 1) * 128],
                        rhs=wp_bf[CK - 1][:, lo:hi],
                        start=False,
                        stop=True,
                    )
                y_sb = stage.tile([128, C], F32, name=f"y_sb{n}", tag="y", bufs=2)
                nc.vector.tensor_add(y_sb[:], y_ps[:], bias_bc[:])
                nc.sync.dma_start(out=out[n * 128:(n + 1) * 128, :], in_=y_sb[:])
        yps.release()


def build_graph():
    nc = bacc.Bacc("TRN2", target_bir_lowering=False, debug=False)
    xT = nc.declare_dram_parameter("xT", [C, N], F32, isOutput=False)
    wqkvT = nc.declare_dram_parameter("wqkvT", [C, 3 * C], F32, isOutput=False)
    wprojT = nc.declare_dram_parameter("wprojT", [C, C], F32, isOutput=False)
    bproj = nc.declare_dram_parameter("bproj", [C], F32, isOutput=False)
    out = nc.declare_dram_parameter("out", [N, C], F32, isOutput=True)
    with tile.TileContext(nc) as tc:
        _emit(tc, xT.ap(), wqkvT.ap(), wprojT.ap(), bproj.ap(), out.ap())
    nc.compile()
    return nc


_GRAPH = None


def _get_graph():
    global _GRAPH
    if _GRAPH is None:
        _GRAPH = build_graph()
    return _GRAPH


def make_in_maps(x, W_qkv, W_proj, b_proj):
    x = np.ascontiguousarray(np.asarray(x, dtype=np.float32))
    wqkvT = np.ascontiguousarray(np.asarray(W_qkv, dtype=np.float32).T)
    wprojT = np.ascontiguousarray(np.asarray(W_proj, dtype=np.float32).T)
    bp = np.ascontiguousarray(np.asarray(b_proj, dtype=np.float32))
    xT_all = np.ascontiguousarray(x.transpose(0, 2, 1))
    return [
        {"xT": xT_all[i], "wqkvT": wqkvT, "wprojT": wprojT, "bproj": bp}
        for i in range(B)
    ]


def run(x, W_qkv, W_proj, b_proj, trace=False):
    nc = _get_graph()
    in_maps = make_in_maps(x, W_qkv, W_proj, b_proj)
    res = run_bass_kernel_spmd(nc, in_maps, core_ids=list(range(B)), trace=trace)
    out = np.stack([res.results[i]["out"] for i in range(B)], axis=0)
    return out.astype(np.float32, copy=False), res


def kernel(x, W_qkv, W_proj, b_proj, H=None, W=None):
    out, _ = run(x, W_qkv, W_proj, b_proj)
    return out


# revision 32
# speedup vs baseline: 1.2347x; 1.0374x over previous
"""Multi-head attention (B=8, N=1024, C=768, 12 heads) on 8 TRN2 NeuronCores.

Sharding: data-parallel over batch — batch element b runs on core b, weights
replicated, zero collectives.

Per-core kernel (all matmuls bf16 on the TensorEngine):
  - Host pre-transposes x, W_qkv, W_proj so every contraction has its
    reduction axis on SBUF partitions; no on-device transposes needed.
  - qkv: q^T,k^T [768,1024] and v [1024,768] via 6-chunk K=768 matmuls.
  - scores are computed TRANSPOSED per head: S^T[k,q] with lhsT=k^T-block,
    rhs=q^T-block, so the exp output P^T feeds the P@V matmul directly as
    the moving operand (no transpose of the attention matrix). The
    1/sqrt(d) scale rides for free on the exp's affine pre-scale.
  - softmax denominators come free: v is stored with a ones-column
    appended per head (lhsT [128,65]); row 64 of the P@V accumulator is
    sum_k exp(S), i.e. the denominator.
  - normalization runs entirely off the TensorEngine's critical path:
    copy the accumulator to SBUF (releasing its PSUM slot), fast
    approximate reciprocal on VectorE, broadcast across partitions on the
    (otherwise idle) GpSimd engine, one elementwise multiply per head.
  - proj: y = attn @ W_proj^T + b_proj, bias added via a K=1 matmul
    (lhsT=ones row, rhs=bias row) accumulated into the same PSUM.
  - qkv chunk emission is interleaved between attention head pairs so the
    TensorEngine stays dense while ScalarE works through the exps.
"""

from contextlib import ExitStack

import numpy as np

import concourse.mybir as mybir
import concourse.tile as tile
from concourse import bacc
from concourse.bass_utils import run_bass_kernel_spmd

B, N, C = 8, 1024, 768
NH, D = 12, 64
CK = C // 128  # 6 contraction chunks of 128
NQ = N // 128  # 8 position chunks of 128
SCALE = D ** -0.5
F32 = mybir.dt.float32
BF16 = mybir.dt.bfloat16
Copy = mybir.ActivationFunctionType.Copy
Exp = mybir.ActivationFunctionType.Exp


def _emit(tc, xT, wqkvT, wprojT, bproj, out):
    nc = tc.nc
    with ExitStack() as ctx:
        sb = ctx.enter_context(tc.tile_pool(name="sb", bufs=1))
        stage = ctx.enter_context(tc.tile_pool(name="stage", bufs=6))
        pp = ctx.enter_context(tc.tile_pool(name="pp", bufs=6))
        small = ctx.enter_context(tc.tile_pool(name="small", bufs=2))
        # PSUM pools are released by hand: qkv+attention use ps/acc, the
        # projection reuses the freed banks for a deeper y pipeline.
        ps = tc.alloc_tile_pool(name="ps", bufs=3, space="PSUM")
        acc = tc.alloc_tile_pool(name="acc", bufs=1, space="PSUM")

        # ---- bias: load row, broadcast across partitions once --------
        bp_row = sb.tile([1, C], F32, name="bp_row", tag="bp_row")
        nc.sync.dma_start(out=bp_row[:], in_=bproj[None, :])
        bias_bc = sb.tile([128, C], F32, name="bias_bc", tag="bias_bc")
        nc.gpsimd.partition_broadcast(bias_bc[:], bp_row[:])

        # ---- load + bf16-convert x^T and W_qkv^T ---------------------
        # The first scores matmul needs all of x^T plus the q-left and
        # k-left weight columns, so those loads are interleaved per
        # c-chunk; v and the right halves follow.
        xT_bf = [
            sb.tile([128, N], BF16, name=f"xT_bf{c}", tag=f"xT_bf{c}")
            for c in range(CK)
        ]
        wq_bf = [
            sb.tile([128, 3 * C], BF16, name=f"wq_bf{c}", tag=f"wq_bf{c}")
            for c in range(CK)
        ]

        def load_w(g, c):
            w_st = stage.tile([128, 384], F32, name=f"w_st{g}_{c}", tag="stage")
            nc.sync.dma_start(
                out=w_st[:],
                in_=wqkvT[c * 128:(c + 1) * 128, g * 384:(g + 1) * 384],
            )
            # convert on ScalarE: it is idle during the qkv phase
            nc.scalar.activation(wq_bf[c][:, g * 384:(g + 1) * 384], w_st[:], Copy)

        def load_x(c, qh):
            x_st = stage.tile([128, 512], F32, name=f"x_st{c}_{qh}", tag="stage")
            nc.sync.dma_start(
                out=x_st[:],
                in_=xT[c * 128:(c + 1) * 128, qh * 512:(qh + 1) * 512],
            )
            nc.vector.tensor_copy(xT_bf[c][:, qh * 512:(qh + 1) * 512], x_st[:])

        for c in range(CK):
            load_x(c, 0)
            load_w(0, c)  # q-left
            load_w(2, c)  # k-left
        for c in range(CK):
            load_x(c, 1)
        for g in (4, 5, 1, 3):  # vL, vR, qR, kR
            for c in range(CK):
                load_w(g, c)

        # ---- qkv projections -----------------------------------------
        # q^T,k^T: chunk m covers rows [m*128,(m+1)*128) of qkv^T;
        # m in 0..5 -> q, m in 6..11 -> k.
        qkT = [
            sb.tile([128, N], BF16, name=f"qkT{m}", tag=f"qkT{m}")
            for m in range(12)
        ]

        def emit_qk(m):
            for qh in range(2):
                qk_ps = ps.tile([128, 512], F32, name=f"qk_ps{m}_{qh}", tag="s")
                for c in range(CK):
                    nc.tensor.matmul(
                        qk_ps[:],
                        lhsT=wq_bf[c][:, m * 128:(m + 1) * 128],
                        rhs=xT_bf[c][:, qh * 512:(qh + 1) * 512],
                        start=(c == 0),
                        stop=(c == CK - 1),
                    )
                nc.vector.tensor_copy(qkT[m][:, qh * 512:(qh + 1) * 512], qk_ps[:])

        # v in natural layout [n, (head, d)] with a ones column appended
        # per head: v_sb[n] is [128, NH, D+1], [:, h, D] == 1.0.
        v_sb = [
            sb.tile([128, NH, D + 1], BF16, name=f"v_sb{n}", tag=f"v_sb{n}")
            for n in range(NQ)
        ]

        def emit_v(n):
            nc.gpsimd.memset(v_sb[n][:, :, D], 1.0)
            for half in range(2):
                v_ps = ps.tile([128, 384], F32, name=f"v_ps{n}_{half}", tag="s")
                for c in range(CK):
                    nc.tensor.matmul(
                        v_ps[:],
                        lhsT=xT_bf[c][:, n * 128:(n + 1) * 128],
                        rhs=wq_bf[c][:, 2 * C + half * 384:2 * C + (half + 1) * 384],
                        start=(c == 0),
                        stop=(c == CK - 1),
                    )
                nc.vector.tensor_copy(
                    v_sb[n][:, half * 6:(half + 1) * 6, 0:D],
                    v_ps[:].rearrange("p (h d) -> p h d", d=D),
                )

        # ---- attention ------------------------------------------------
        attn_bf = [
            sb.tile([128, N], BF16, name=f"attn_bf{p}", tag=f"attn_bf{p}")
            for p in range(6)
        ]

        def emit_head(h, filler=None):
            """S^T + exp + P@V for head h; `filler` emits extra PE work
            early in the stream (previous head's deferred normalize, next
            qkv chunk) so PE has exp-independent work while ScalarE runs."""
            q_tile = qkT[h // 2]
            k_tile = qkT[6 + h // 2]
            ro = (h % 2) * 64
            out_aug = acc.tile([D + 1, N], F32, name=f"oaug{h}", tag="acc")

            def emit_S(kc):
                st = ps.tile([128, N], F32, name=f"s{h}_{kc}", tag="s")
                for qh in range(2):
                    nc.tensor.matmul(
                        st[:, qh * 512:(qh + 1) * 512],
                        lhsT=k_tile[ro:ro + D, kc * 128:(kc + 1) * 128],
                        rhs=q_tile[ro:ro + D, qh * 512:(qh + 1) * 512],
                        start=True,
                        stop=True,
                    )
                pt = pp.tile([128, N], BF16, name=f"P{h}_{kc}", tag="P")
                nc.scalar.activation(pt[:], st[:], Exp, scale=SCALE)
                return pt

            def emit_v_mm(kc, pt):
                for qh in range(2):
                    nc.tensor.matmul(
                        out_aug[:, qh * 512:(qh + 1) * 512],
                        lhsT=v_sb[kc][:, h, :],
                        rhs=pt[:, qh * 512:(qh + 1) * 512],
                        start=(kc == 0),
                        stop=(kc == NQ - 1),
                    )

            # software pipeline: exp(kc) overlaps S(kc+1) and P@V(kc-1)
            pts = {0: emit_S(0), 1: emit_S(1)}
            if filler is not None:
                filler()
            for kc in range(NQ):
                emit_v_mm(kc, pts.pop(kc))
                if kc + 2 < NQ:
                    pts[kc + 2] = emit_S(kc + 2)
            return out_aug

        def emit_norm_pre(h, oa, direct=False):
            """DVE/GpSimd-only part: reciprocal chain first (it gates the
            final multiply), then stage the accumulator to SBUF to release
            its PSUM slot. For the last head (`direct`) the multiply reads
            the accumulator straight from PSUM instead — shortest tail."""
            if not direct:
                # staging copy FIRST: it releases the single-slot PSUM
                # accumulator, which gates the next head's P@V matmuls
                un = small.tile([D, N], F32, name=f"un{h}", tag="un")
                nc.vector.tensor_copy(un[:], oa[0:D, :])
            dn = small.tile([1, N], F32, name=f"dn{h}", tag="dn")
            nc.vector.tensor_copy(dn[:], oa[D:D + 1, :])
            rc = small.tile([1, N], F32, name=f"rc{h}", tag="rc")
            # reciprocal_approx_fast's uOp program only works from
            # partition 0 on hardware, hence the dn bounce copy above.
            nc.vector.reciprocal_approx_fast(rc[:], dn[:])
            rcb = small.tile([1, N], BF16, name=f"rcb{h}", tag="rcb")
            nc.vector.tensor_copy(rcb[:], rc[:])
            bcast = small.tile([64, N], BF16, name=f"bcast{h}", tag="bcast")
            nc.gpsimd.partition_broadcast(bcast[:], rcb[:])
            if direct:
                return oa, bcast
            return un, bcast

        def emit_norm_post(h, un, bcast):
            p, ro = h // 2, (h % 2) * 64
            nc.vector.tensor_mul(attn_bf[p][ro:ro + 64, :], un[0:D, :], bcast[:])

        emit_qk(0)
        emit_qk(6)
        emit_v(0)

        # Remaining qkv work rides inside the attention stream as PE
        # filler during exp waits: head 0 carries the other v chunks
        # (needed from its own P@V loop onward), later heads each carry
        # one q/k chunk, landing one pair ahead of first use.
        QK_FILL = {1: (1, 7), 2: (2,), 3: (8,), 4: (3,), 5: (9,),
                   6: (4,), 7: (10,), 8: (5,), 9: (11,)}
        pending = None
        for h in range(NH):
            fillers = []
            if h == 0:
                fillers.append(lambda: [emit_v(n) for n in range(1, NQ)])
            for m in QK_FILL.get(h, ()):
                fillers.append(lambda m=m: emit_qk(m))
            if pending is not None:
                ph, un, bc = pending
                fillers.append(lambda ph=ph, un=un, bc=bc: emit_norm_post(ph, un, bc))

            def filler():
                for f in fillers:
                    f()

            oa = emit_head(h, filler=filler)
            pending = (h, *emit_norm_pre(h, oa))
        emit_norm_post(*pending)

        # ---- output projection ---------------------------------------
        acc.release()
        ps.release()
        yps = tc.alloc_tile_pool(name="yps", bufs=4, space="PSUM")
        wp_bf = []
        for c in range(CK):
            wp_st = stage.tile([128, C], F32, name=f"wp_st{c}", tag="stage")
            nc.sync.dma_start(out=wp_st[:], in_=wprojT[c * 128:(c + 1) * 128, :])
            t = sb.tile([128, C], BF16, name=f"wp_bf{c}", tag=f"wp_bf{c}")
            nc.vector.tensor_copy(t[:], wp_st[:])
            wp_bf.append(t)

        for n in range(NQ):
            y_ps = yps.tile([128, C], F32, name=f"y_ps{n}", tag="y_ps")
            for lo, hi in ((0, 512), (512, 768)):
                for c in range(CK):
                    nc.tensor.matmul(
                        y_ps[:, lo:hi],
                        lhsT=attn_bf[c][:, n * 128:(n + 1) * 128],
                        rhs=wp_bf[c][:, lo:hi],
                        start=(c == 0),
                        stop=(c == CK - 1),
                    )
            y_sb = stage.tile([128, C], F32, name=f"y_sb{n}", tag="y", bufs=2)
            nc.vector.tensor_add(y_sb[:], y_ps[:], bias_bc[:])
            nc.sync.dma_start(out=out[n * 128:(n + 1) * 128, :], in_=y_sb[:])
        yps.release()


def build_graph():
    nc = bacc.Bacc("TRN2", target_bir_lowering=False, debug=False)
    xT = nc.declare_dram_parameter("xT", [C, N], F32, isOutput=False)
    wqkvT = nc.declare_dram_parameter("wqkvT", [C, 3 * C], F32, isOutput=False)
    wprojT = nc.declare_dram_parameter("wprojT", [C, C], F32, isOutput=False)
    bproj = nc.declare_dram_parameter("bproj", [C], F32, isOutput=False)
    out = nc.declare_dram_parameter("out", [N, C], F32, isOutput=True)
    with tile.TileContext(nc) as tc:
        _emit(tc, xT.ap(), wqkvT.ap(), wprojT.ap(), bproj.ap(), out.ap())
    nc.compile()
    return nc


_GRAPH = None


def _get_graph():
    global _GRAPH
    if _GRAPH is None:
        _GRAPH = build_graph()
    return _GRAPH


def make_in_maps(x, W_qkv, W_proj, b_proj):
    x = np.ascontiguousarray(np.asarray(x, dtype=np.float32))
    wqkvT = np.ascontiguousarray(np.asarray(W_qkv, dtype=np.float32).T)
    wprojT = np.ascontiguousarray(np.asarray(W_proj, dtype=np.float32).T)
    bp = np.ascontiguousarray(np.asarray(b_proj, dtype=np.float32))
    xT_all = np.ascontiguousarray(x.transpose(0, 2, 1))
    return [
        {"xT": xT_all[i], "wqkvT": wqkvT, "wprojT": wprojT, "bproj": bp}
        for i in range(B)
    ]


def run(x, W_qkv, W_proj, b_proj, trace=False):
    nc = _get_graph()
    in_maps = make_in_maps(x, W_qkv, W_proj, b_proj)
    res = run_bass_kernel_spmd(nc, in_maps, core_ids=list(range(B)), trace=trace)
    out = np.stack([res.results[i]["out"] for i in range(B)], axis=0)
    return out.astype(np.float32, copy=False), res


def kernel(x, W_qkv, W_proj, b_proj, H=None, W=None):
    out, _ = run(x, W_qkv, W_proj, b_proj)
    return out
